# revision 1
# baseline (speedup 1.0000x reference)
"""GAT attention head (gnn_message_passing) on 8 trn2 NeuronCores.

Strategy (edge-parallel, 1D graph partitioning by dst):
  - Shard dst nodes into 8 contiguous ranges of 6250; each core owns all
    edges pointing into its range, so segment softmax/aggregation is local.
  - Host-side index prep only: bucket edges by dst shard, degree-sort each
    shard's dsts, lay edges out as [128-dst block x S_b slot] rectangles.
  - Phase 1 (device, replicated): h_aug table [nodes, 64] f32 where
    cols 0:32 = inputs @ W.T, col 32 = inputs @ (W.T a_src_w),
    col 33 = inputs @ (W.T a_dst_w). PE matmul from host-transposed inputs.
  - Phase 2 (device): per block one dma_gather of 512B node-PAIR rows
    (idx = src >> 1, int16-safe; parity src & 1 selects the half), then
    dense vector ops: score = lrelu(e_dst + e_src + b), exp (softmax-max
    subtraction dropped -- mathematically exact, f32-safe at this scale),
    slot-reduce of exp and exp*h, normalize, ELU.
  Padding slots point at a sentinel node row with e_src = -1e30 -> exp = 0.
"""

import math

import numpy as np

import concourse.bass as bass
import concourse.tile as tile
from concourse import bacc
from concourse import mybir
from concourse.bass_utils import run_bass_kernel_spmd
from concourse.vector_clock import ScopedClock

F32 = mybir.dt.float32
I16 = mybir.dt.int16
AF = mybir.ActivationFunctionType
X = mybir.AxisListType.X

P = 128
IN_DIM = 256
OUT_DIM = 32
TCOLS = 64  # f32 per node row in the h_aug table (256B; pair row = 512B)
SLABW = 2048  # nodes per phase-1 slab
N_CORES = 8

# ---------------------------------------------------------------------------
# Walrus in this container rejects instructions with more than ~2 sem waits
# ("Too many sync wait commands").  TileContext's kernel-tail drain carries
# one wait per tile semaphore; spread them over single-wait NOPs instead.
_PATCHED = False


def _patched_drain_and_barrier(self, tick_clock, wait_clock):
    nc = self.nc
    probe = nc.sync.nop(nofuse=True, hint="drain_waits")
    wait_clock.add_sem_waits(probe.ins, ScopedClock({None: tick_clock.global_clock}))
    si = probe.ins.sync_info
    waits = list(si.on_wait) if si is not None else []
    if si is not None:
        si.on_wait = waits[:1]
    for i in range(1, len(waits)):
        nop = nc.sync.nop(nofuse=True, hint=f"drain_waits_{i}")
        if nop.ins.sync_info is None:
            nop.ins.sync_info = mybir.SyncInfo(on_wait=[], on_update=[])
        nop.ins.sync_info.on_wait = [waits[i]]
    nc.sync.drain()
    nc.all_engine_barrier()
    assert self.sems is not None
    popped = nc._tile_sem_poison_stack.pop()
    assert popped is self._sem_poison
    nc.clear_and_free_semaphores(list(self.sems.allocated().values()))
    nc.all_engine_barrier()


def _install_patch():
    global _PATCHED
    if not _PATCHED:
        tile.TileContext._drain_and_barrier = _patched_drain_and_barrier
        _PATCHED = True


# ---------------------------------------------------------------------------
# Host-side sharding / index layout


def plan(inputs, edge_src, edge_dst, W, a_dst_w, a_dst_b, a_src_w, a_src_b,
         n_cores=N_CORES):
    N = inputs.shape[0]
    assert N % n_cores == 0
    shard = N // n_cores
    nblk = (shard + P - 1) // P
    nloc = nblk * P
    sent = N  # sentinel node id (even; parity 0)
    nodes_p = ((N + 1 + P - 1) // P) * P  # matmul-padded node count
    assert nodes_p % P == 0 and nodes_p // 2 <= 32768

    # inputs transposed + zero-padded; W folded with the attention vectors.
    xT = np.zeros((IN_DIM, nodes_p), np.float32)
    xT[:, :N] = np.asarray(inputs, np.float32).T
    waug = np.zeros((IN_DIM, TCOLS), np.float32)
    waug[:, :OUT_DIM] = np.asarray(W, np.float32).T
    waug[:, OUT_DIM] = np.asarray(W, np.float32).T @ np.asarray(a_src_w, np.float32)
    waug[:, OUT_DIM + 1] = np.asarray(W, np.float32).T @ np.asarray(a_dst_w, np.float32)
    bconst = float(np.float32(a_src_b) + np.float32(a_dst_b))

    edge_src = np.asarray(edge_src)
    edge_dst = np.asarray(edge_dst)
    core_of = edge_dst // shard

    orders, ranks, degss = [], [], []
    S_blocks = np.zeros(nblk, np.int64)
    for k in range(n_cores):
        m = core_of == k
        ed = edge_dst[m] - k * shard
        deg = np.bincount(ed, minlength=shard)
        order = np.argsort(-deg, kind="stable")
        rank = np.empty(shard, np.int64)
        rank[order] = np.arange(shard)
        degs = deg[order]
        orders.append(order)
        ranks.append(rank)
        degss.append(degs)
        for b in range(nblk):
            S_blocks[b] = max(S_blocks[b], degs[b * P])
    S_blocks = np.maximum(S_blocks, 1)

    Pb = (S_blocks + 1) * P  # positions per block (col 0 = dst pair rows)
    off = np.zeros(nblk + 1, np.int64)
    off[1:] = np.cumsum(Pb)
    total = int(off[-1])
    c16 = total // 16
    cp = total // P

    idx_grids, par_grids = [], []
    for k in range(n_cores):
        m = core_of == k
        es = edge_src[m]
        ed = edge_dst[m] - k * shard
        flat_idx = np.full(total, sent >> 1, np.int16)
        flat_par = np.zeros(total, np.float32)
        # col 0 of each block: the block's own dst node rows (for e_dst)
        loc = np.concatenate([orders[k], np.full(nloc - shard, sent, np.int64)])
        gnode = np.where(loc < shard, k * shard + loc, sent)
        for b in range(nblk):
            pos0 = off[b]
            blk_nodes = gnode[b * P:(b + 1) * P]
            flat_idx[pos0:pos0 + P] = (blk_nodes >> 1).astype(np.int16)
            flat_par[pos0:pos0 + P] = (blk_nodes & 1).astype(np.float32)
        # edge slots
        r = ranks[k][ed]
        o2 = np.argsort(r, kind="stable")
        rs = r[o2]
        ss = es[o2]
        slot = np.arange(len(rs)) - np.searchsorted(rs, rs)
        b = rs // P
        pos = off[b] + (slot + 1) * P + (rs % P)
        flat_idx[pos] = (ss >> 1).astype(np.int16)
        flat_par[pos] = (ss & 1).astype(np.float32)
        idx_grids.append(np.tile(flat_idx.reshape(c16, 16).T, (8, 1)))
        par_grids.append(np.ascontiguousarray(
            flat_par.reshape(cp, P).T))

    return dict(
        N=N, shard=shard, nblk=nblk, nloc=nloc, nodes_p=nodes_p, sent=sent,
        S=[int(s) for s in S_blocks], off=off, c16=c16, cp=cp,
        xT=xT, waug=waug, bconst=bconst,
        idx=idx_grids, par=par_grids, orders=orders, n_cores=n_cores,
    )


# ---------------------------------------------------------------------------
# Device program


def build_nc(pl):
    _install_patch()
    nodes_p, nblk, c16, cp = pl["nodes_p"], pl["nblk"], pl["c16"], pl["cp"]
    S_list, off, nloc, bconst = pl["S"], pl["off"], pl["nloc"], pl["bconst"]
    smax = max(S_list)

    nc = bacc.Bacc()
    xT = nc.dram_tensor("xT", [IN_DIM, nodes_p], F32, kind="ExternalInput")
    waug = nc.dram_tensor("waug", [IN_DIM, TCOLS], F32, kind="ExternalInput")
    idx_d = nc.dram_tensor("idx", [P, c16], I16, kind="ExternalInput")
    par_d = nc.dram_tensor("par", [P, cp], F32, kind="ExternalInput")
    out_d = nc.dram_tensor("out", [nloc, OUT_DIM], F32, kind="ExternalOutput")
    table = nc.dram_tensor("table", [nodes_p, TCOLS], F32)
    tpairs = table[:, :].rearrange("(r two) c -> r (two c)", two=2)

    with tile.TileContext(nc) as tc:
        with (
            tc.tile_pool(name="const", bufs=1) as const,
            tc.tile_pool(name="xslab", bufs=2) as xpool,
            tc.tile_pool(name="stage", bufs=2) as stpool,
            tc.tile_pool(name="psum", bufs=4, space="PSUM") as psum,
            tc.tile_pool(name="gath", bufs=2) as gpool,
            tc.tile_pool(name="med", bufs=2) as med,
            tc.tile_pool(name="tmp", bufs=2) as tmppool,
            tc.tile_pool(name="small", bufs=4) as small,
        ):
            wa = const.tile([P, 2, TCOLS], F32)
            nc.sync.dma_start(out=wa[:, 0, :], in_=waug[0:P, :])
            nc.sync.dma_start(out=wa[:, 1, :], in_=waug[P:2 * P, :])
            idx_t = const.tile([P, c16], I16)
            nc.sync.dma_start(out=idx_t[:], in_=idx_d[:, :])
            par_t = const.tile([P, cp], F32)
            nc.sync.dma_start(out=par_t[:], in_=par_d[:, :])
            sentv = const.tile([1, 1], F32)
            nc.vector.memset(sentv[:], -1.0e30)

            # ---- phase 1: h_aug table ----
            nslab = math.ceil(nodes_p / SLABW)
            for s in range(nslab):
                c0 = s * SLABW
                w = min(SLABW, nodes_p - c0)
                nsub = w // P
                xa = xpool.tile([P, 2, SLABW], F32, tag="xa")
                nc.sync.dma_start(out=xa[:, 0, :w], in_=xT[0:P, c0:c0 + w])
                nc.sync.dma_start(out=xa[:, 1, :w], in_=xT[P:2 * P, c0:c0 + w])
                st = stpool.tile([P, SLABW // P, TCOLS], F32, tag="st")
                if True:  # keep indentation stable
                    for nt in range(nsub):
                        ps = psum.tile([P, TCOLS], F32)
                        nc.tensor.matmul(
                            out=ps[:], lhsT=xa[:, 0, nt * P:(nt + 1) * P],
                            rhs=wa[:, 0, :], start=True, stop=False)
                        nc.tensor.matmul(
                            out=ps[:], lhsT=xa[:, 1, nt * P:(nt + 1) * P],
                            rhs=wa[:, 1, :], start=False, stop=True)
                        nc.scalar.copy(st[:, nt, :], ps[:])
                nc.sync.dma_start(
                    out=table[c0:c0 + w, :].rearrange("(nb p) c -> p nb c", p=P),
                    in_=st[:, :nsub, :])
            # sentinel e_src = -1e30
            nc.sync.dma_start(
                out=table[pl["sent"]:pl["sent"] + 1, OUT_DIM:OUT_DIM + 1],
                in_=sentv[:1, :1])

            # ---- phase 2: per-block gather + dense softmax/aggregate ----
            for b in range(nblk):
                S = S_list[b]
                Pb = (S + 1) * P
                o16 = int(off[b]) // 16
                ocp = int(off[b]) // P

                G = gpool.tile([P, smax + 1, P], F32, tag="G")
                MAXI = 4096
                done = 0
                while done < Pb:
                    ch = min(MAXI, Pb - done)
                    nc.gpsimd.dma_gather(
                        out_ap=G[:, done // P:(done + ch) // P, :], in_ap=tpairs,
                        idxs_ap=idx_t[:, o16 + done // 16:o16 + (done + ch) // 16],
                        num_idxs=ch, num_idxs_reg=ch, elem_size=P,
                        single_packet=False)
                    done += ch

                par0 = par_t[:, ocp:ocp + 1]
                parS = par_t[:, ocp + 1:ocp + 1 + S]

                # e_dst for the block's 128 dsts (parity select + bias)
                ed = small.tile([P, 1], F32, tag="ed")
                nc.vector.tensor_sub(ed[:], G[:, 0, 97:98], G[:, 0, 33:34])
                nc.vector.tensor_mul(ed[:], ed[:], par0)
                nc.vector.tensor_add(ed[:], ed[:], G[:, 0, 33:34])
                nc.vector.tensor_scalar_add(ed[:], ed[:], bconst)

                # e_src per slot (parity select), score, exp
                eslo = G[:, 1:S + 1, 32:33].rearrange("p s o -> p (s o)")
                eshi = G[:, 1:S + 1, 96:97].rearrange("p s o -> p (s o)")
                ex = med.tile([P, smax], F32, tag="ex")
                exS = ex[:, :S]
                nc.vector.tensor_sub(exS, eshi, eslo)
                nc.vector.tensor_mul(exS, exS, parS)
                nc.vector.tensor_add(exS, exS, eslo)
                nc.vector.tensor_scalar_add(exS, exS, ed[:])
                nc.vector.scalar_tensor_tensor(
                    out=exS, in0=exS, scalar=0.2, in1=exS,
                    op0=mybir.AluOpType.mult, op1=mybir.AluOpType.max)
                nc.scalar.activation(exS, exS, AF.Exp)

                sw = small.tile([P, 1], F32, tag="sw")
                nc.vector.reduce_sum(out=sw[:], in_=exS, axis=X)

                whi = med.tile([P, smax], F32, tag="whi")
                nc.vector.tensor_mul(whi[:, :S], exS, parS)
                wlo = med.tile([P, smax], F32, tag="wlo")
                nc.vector.tensor_sub(wlo[:, :S], exS, whi[:, :S])

                # weighted message accumulation: tmp[c, 0:S] = wlo*h_lo,
                # tmp[c, S:2S] = whi*h_hi, then one reduce over 2S.
                tmp = tmppool.tile([P, OUT_DIM, 2 * smax], F32, tag="tmp")
                glo = G[:, 1:S + 1, 0:OUT_DIM].rearrange("p s c -> p c s")
                ghi = G[:, 1:S + 1, TCOLS:TCOLS + OUT_DIM].rearrange("p s c -> p c s")
                wlo_b = wlo[:, None, :S].broadcast_to([P, OUT_DIM, S])
                whi_b = whi[:, None, :S].broadcast_to([P, OUT_DIM, S])
                nc.vector.tensor_tensor(
                    out=tmp[:, :, 0:S], in0=glo, in1=wlo_b, op=mybir.AluOpType.mult)
                nc.vector.tensor_tensor(
                    out=tmp[:, :, S:2 * S], in0=ghi, in1=whi_b,
                    op=mybir.AluOpType.mult)
                acc = small.tile([P, OUT_DIM], F32, tag="acc")
                nc.vector.reduce_sum(out=acc[:], in_=tmp[:, :, 0:2 * S], axis=X)

                # normalize + ELU
                nc.vector.tensor_scalar_max(sw[:], sw[:], 1.0e-12)
                nc.vector.reciprocal(sw[:], sw[:])
                nc.vector.tensor_scalar_mul(acc[:], acc[:], sw[:])
                pos = small.tile([P, OUT_DIM], F32, tag="pos")
                nc.vector.tensor_scalar_max(pos[:], acc[:], 0.0)
                nc.vector.tensor_scalar_min(acc[:], acc[:], 0.0)
                nc.scalar.activation(acc[:], acc[:], AF.Exp)
                nc.vector.tensor_add(acc[:], acc[:], pos[:])
                nc.vector.tensor_scalar_add(acc[:], acc[:], -1.0)
                nc.sync.dma_start(out=out_d[b * P:(b + 1) * P, :], in_=acc[:])
    nc.finalize()
    return nc


# ---------------------------------------------------------------------------


def _in_maps(pl):
    return [
        {"xT": pl["xT"], "waug": pl["waug"], "idx": pl["idx"][k],
         "par": pl["par"][k]}
        for k in range(pl["n_cores"])
    ]


def unshard(pl, results):
    N, shard = pl["N"], pl["shard"]
    full = np.zeros((N, OUT_DIM), np.float32)
    for k in range(pl["n_cores"]):
        ok = results[k]["out"]
        full[k * shard + pl["orders"][k]] = ok[:shard]
    return full


def kernel(**inputs):
    pl = plan(inputs["inputs"], inputs["edge_src"], inputs["edge_dst"],
              inputs["W"], inputs["a_dst_w"], inputs["a_dst_b"],
              inputs["a_src_w"], inputs["a_src_b"])
    nc = build_nc(pl)
    res = run_bass_kernel_spmd(nc, _in_maps(pl),
                               core_ids=list(range(pl["n_cores"])))
    return unshard(pl, res.results)

